# revision 45
# baseline (speedup 1.0000x reference)
"""MLA-style attention kernel for 8 TRN2 NeuronCores.

Sharding: core c -> batch b = c//4, heads r*4..r*4+3 where r = c%4.
Each core recomputes the full latent down-projection for its batch
locally (cheaper than the ~126us AllGather the cost model charges),
then computes its 4 heads' attention and a partial output projection
summed on the host.

All layout work (x^T, every weight transpose, planar rope reordering,
cos/sin table duplication, bf16 casts) happens on the host in numpy, so
the device runs pure matmul/softmax work.  The main loop is braided
across 512-row T-chunks: the latency-bound attention row tq=c is
emitted interleaved into chunk c+1's dense down-projection matmul
groups, so PE never drains while masks (DVE) and exp (ACT) catch up;
up-projection and output projection sections separate the other
cross-engine dependencies.  Everything stays SBUF-resident; outputs are
written bf16 and upcast on the host.

Per-block attention: one bf16 content matmul (128-dim) plus one bf16
rope matmul (64-dim) accumulate S^T [k,q] in PSUM; diagonal blocks are
q-sliced (w = 512-128*ks) and get a 128-column sliding-mask add; exp
writes bf16 P^T tiles consumed by a ones-column denominator matmul and
the PV matmul (8-deep software pipeline across two interleaved heads,
both heads' denominators packed into one PSUM bank).  Normalization
runs entirely off the PE: reciprocal (DVE) -> partition_broadcast
(Pool) -> multiply (DVE).  v is computed directly in natural [t, hs]
layout for all 4 heads in one matmul group.  q_r is computed flipped
([t, rope] orientation, all heads batched: 64-row instead of 512-row
passes), roped with six 128-wide DVE ops, and transposed back with four
cheap PE transposes per head.  A short junk matmul chain on the
yet-unloaded weight tile warms the PE clock-gate ramp through the first
DMA window.
"""
import math
import numpy as np

import concourse.bass as bass
import concourse.bacc as bacc
import concourse.mybir as mybir
import concourse.tile as tile
from concourse.bass_utils import run_bass_kernel_spmd

F32 = mybir.dt.float32
F32R = mybir.dt.float32r
BF16 = mybir.dt.bfloat16
Exp = mybir.ActivationFunctionType.Exp

B, T, C = 2, 2048, 2048
H = 16
HS = 128
NL = 512
RHD = 64
HLOC = 4              # heads per core
P = 128
NNL = NL // P         # 4 latent part-tiles
NCI = C // P          # 16 c part-tiles
TCH = 512
NCH = T // TCH        # 4 chunks of T
SCALE = 1.0 / math.sqrt(HS + RHD)
NEG = -1.0e30

_NC_CACHE = {}


def build():
    nc = bacc.Bacc("TRN2", target_bir_lowering=False, debug=False, num_devices=8)

    xT_ext = nc.dram_tensor("xT", [C, T], BF16, kind="ExternalInput")
    wdqT_ext = nc.dram_tensor("wdqT", [C, NL], BF16, kind="ExternalInput")
    wdkvT_ext = nc.dram_tensor("wdkvT", [C, NL], BF16, kind="ExternalInput")
    wkrT_ext = nc.dram_tensor("wkrT", [C, RHD], BF16, kind="ExternalInput")
    wuqT_ext = nc.dram_tensor("wuqT", [NL, HLOC * HS], BF16, kind="ExternalInput")
    wukT_ext = nc.dram_tensor("wukT", [NL, HLOC * HS], BF16, kind="ExternalInput")
    wuvT_ext = nc.dram_tensor("wuvT", [NL, HLOC * HS], BF16, kind="ExternalInput")
    wqrT_ext = nc.dram_tensor("wqrT", [NL, HLOC * RHD], BF16, kind="ExternalInput")
    woT_ext = nc.dram_tensor("woT", [HLOC * HS, C], BF16, kind="ExternalInput")
    ca_ext = nc.dram_tensor("ca", [RHD, T], BF16, kind="ExternalInput")
    sa_ext = nc.dram_tensor("sa", [RHD, T], BF16, kind="ExternalInput")
    out_ext = nc.dram_tensor("out", [C, T], BF16, kind="ExternalOutput")

    bfnp = mybir.dt.np(BF16)
    onesbf_dram = nc.inline_tensor(np.ones((P, 1), dtype=bfnp), name="onesbfc")
    # transposed sliding causal mask for S^T tiles [k-sub, q]:
    # m[jj, 384 + u] = 0 iff u >= jj (else -1e30).  For the diagonal
    # kc == tq with q-slice starting at 128*ks, the slice [384:384+w]
    # allows q - 128*ks >= jj for every ks.
    m = np.full((P, 896), NEG, dtype=np.float32)
    for jj in range(P):
        m[jj, 384 + jj:] = 0.0
    masks_dram = nc.inline_tensor(m.astype(bfnp), name="maskc")

    def ci_fold(ext, width):
        """DRAM [n*128, width] viewed as [128, n, width] (n part-tiles
        stacked along the free dim)."""
        return ext.ap().rearrange("(n p) w -> p n w", p=P)

    def fold_dst(t, width):
        return t[:].rearrange("p (n w) -> p n w", w=width)

    with tile.TileContext(nc) as tc:
        with (
            tc.tile_pool(name="pers", bufs=1) as pers,
            tc.tile_pool(name="pwork", bufs=1) as pwork,
            tc.tile_pool(name="pmm", bufs=4, space="PSUM") as pmm,
            tc.tile_pool(name="pou", bufs=2, space="PSUM") as pou,
        ):
            # ---------------- persistent weights / tables ----------------
            # Load order is startup-latency-critical: interleave the x^T
            # chunk-0 pieces with W_dq pieces so the first down-proj matmul
            # can start after ~1MB of DMA, not after every weight.
            wdq = pers.tile([P, NCI * NL], BF16, tag="wdq", name="wdq")
            wdkv = pers.tile([P, NCI * NL], BF16, tag="wdkv", name="wdkv")
            wkr = pers.tile([P, NCI * RHD], BF16, tag="wkr", name="wkr")
            wuq = pers.tile([P, NNL * HLOC * HS], BF16, tag="wuq", name="wuq")
            wuk = pers.tile([P, NNL * HLOC * HS], BF16, tag="wuk", name="wuk")
            wuv = pers.tile([P, NNL * HLOC * HS], BF16, tag="wuv", name="wuv")
            wqr = pers.tile([P, NNL * HLOC * RHD], BF16, tag="wqr", name="wqr")
            wo = pers.tile([P, HLOC * C], BF16, tag="wo", name="wo")
            onesbf = pers.tile([P, 1], BF16, tag="onesbf", name="onesbf")
            maskbuf = pers.tile([P, 896], BF16, tag="maskbuf", name="maskbuf")
            ca = pers.tile([RHD, T], BF16, tag="ca", name="ca")
            sa = pers.tile([RHD, T], BF16, tag="sa", name="sa")

            def load_piece(t, ext, width, pc, npc=4):
                """Load part-tile group pc (of npc) of a folded weight."""
                n = ext.shape[0] // P
                lo, hi = pc * n // npc, (pc + 1) * n // npc
                nc.sync.dma_start(
                    fold_dst(t, width)[:, lo:hi, :],
                    ext.ap()[lo * P:hi * P, :].rearrange(
                        "(n p) w -> p n w", p=P),
                )

            def load_rest():
                nc.sync.dma_start(wkr[:].rearrange("p (n w) -> p n w", w=RHD),
                                  ci_fold(wkrT_ext, RHD))
                nc.sync.dma_start(ca[:], ca_ext.ap())
                nc.sync.dma_start(sa[:], sa_ext.ap())
                nc.sync.dma_start(onesbf[:], onesbf_dram.ap())
                nc.sync.dma_start(maskbuf[:], masks_dram.ap())
                for t, ext, width in (
                    (wuq, wuqT_ext, HLOC * HS),
                    (wuk, wukT_ext, HLOC * HS),
                    (wuv, wuvT_ext, HLOC * HS),
                    (wqr, wqrT_ext, HLOC * RHD),
                    (wo, woT_ext, C),
                ):
                    nc.sync.dma_start(fold_dst(t, width),
                                      ci_fold(ext, width))

            # persistent per-head K/V state + shared rope key
            kcT = [pers.tile([P, T], BF16, tag=f"kcT{h}", name=f"kcT{h}")
                   for h in range(HLOC)]
            vv = [pers.tile([P, T], BF16, tag=f"vv{h}", name=f"vv{h}")
                  for h in range(HLOC)]
            kr = pers.tile([RHD, T], BF16, tag="kr", name="kr")

            def rope(dst, dst_sl, raw, tmp, sl):
                """dst[:, dst_sl] = rope(raw) with planar re/im halves."""
                nc.vector.tensor_mul(tmp[0:32, :], raw[32:64, :], sa[32:64, sl])
                nc.vector.tensor_mul(tmp[32:64, :], raw[32:64, :], ca[32:64, sl])
                nc.vector.tensor_mul(dst[0:32, dst_sl], raw[0:32, :], ca[0:32, sl])
                nc.vector.tensor_mul(dst[32:64, dst_sl], raw[0:32, :], sa[0:32, sl])
                nc.vector.tensor_sub(
                    dst[0:32, dst_sl], dst[0:32, dst_sl], tmp[0:32, :]
                )
                nc.vector.tensor_add(
                    dst[32:64, dst_sl], dst[32:64, dst_sl], tmp[32:64, :]
                )

            # ---------------- chunk-pipelined main loop -------------------
            # Emission is braided across chunks so that the latency-bound
            # attention row tq=c executes while the dense down-proj of
            # chunk c+1 keeps every other engine's queue drained:
            #   dp(0) up(0) [dp(1) attn(0) up(1) out(0)] [dp(2) attn(1) ...
            st = [dict() for _ in range(NCH)]

            def emit_dp(c):
                tsl = slice(c * TCH, (c + 1) * TCH)
                # prefetch x^T for chunk c+1 one full section group ahead
                if c == 0:
                    # junk matmuls on the not-yet-loaded weight tile keep
                    # the PE busy through the first DMA window so the
                    # clock-gate ramp is warm when real matmuls start
                    for _ in range(5):
                        warm = pmm.tile([P, 256], F32, tag="mm", name="mm")
                        nc.tensor.matmul(warm[:], wdq[:, 0:P],
                                         wdq[:, 0:256],
                                         start=True, stop=True,
                                         skip_group_check=True)
                if c == 0:
                    st[0]["xt"] = pwork.tile([P, NCI * TCH], BF16, tag="xt",
                                             bufs=2, name="xt")
                    for pc in range(8):
                        lo, hi = pc * 2, (pc + 1) * 2
                        nc.sync.dma_start(
                            fold_dst(st[0]["xt"], TCH)[:, lo:hi, :],
                            xT_ext.ap()[lo * P:hi * P, 0:TCH].rearrange(
                                "(n p) w -> p n w", p=P),
                        )
                        load_piece(wdq, wdqT_ext, NL, pc, npc=8)
                    for pc in range(4):
                        load_piece(wdkv, wdkvT_ext, NL, pc)
                    load_rest()
                if c + 1 < NCH:
                    nxt = pwork.tile([P, NCI * TCH], BF16, tag="xt", bufs=2,
                                     name="xt")
                    st[c + 1]["xt"] = nxt
                    nc.sync.dma_start(
                        fold_dst(nxt, TCH),
                        xT_ext.ap()[:, (c + 1) * TCH:(c + 2) * TCH].rearrange(
                            "(n p) w -> p n w", p=P),
                    )
                xt = st[c]["xt"]
                cq_sb = [pwork.tile([P, TCH], BF16, tag=f"cq{g}", bufs=1,
                                    name=f"cq{g}") for g in range(NNL)]
                ckv_sb = [pwork.tile([P, TCH], BF16, tag=f"ckv{g}", bufs=1,
                                     name=f"ckv{g}") for g in range(NNL)]
                st[c]["cq"], st[c]["ckv"] = cq_sb, ckv_sb
                for w_sb, dst in ((wdq, cq_sb), (wdkv, ckv_sb)):
                    for g in range(NNL):
                        acc = pmm.tile([P, TCH], F32, tag="mm", name="mm")
                        for ci in range(NCI):
                            nc.tensor.matmul(
                                acc[:],
                                w_sb[:, ci * NL + g * P: ci * NL + (g + 1) * P],
                                xt[:, ci * TCH:(ci + 1) * TCH],
                                start=(ci == 0),
                                stop=(ci == NCI - 1),
                            )
                        cp = nc.scalar.copy if g % 2 == 0 else nc.vector.tensor_copy
                        cp(dst[g][:], acc[:])
                acck = pmm.tile([P, TCH], F32, tag="mm", name="mm")
                for ci in range(NCI):
                    nc.tensor.matmul(
                        acck[0:RHD, :],
                        wkr[:, ci * RHD:(ci + 1) * RHD],
                        xt[:, ci * TCH:(ci + 1) * TCH],
                        start=(ci == 0),
                        stop=(ci == NCI - 1),
                    )
                rtmp = pwork.tile([RHD, TCH], F32, tag="rtmp", name="rtmp")
                rope(kr, tsl, acck[0:RHD, :], rtmp, tsl)

            def emit_up(c):
                tsl = slice(c * TCH, (c + 1) * TCH)
                cq_sb, ckv_sb = st[c]["cq"], st[c]["ckv"]
                qc_loc = [pwork.tile([P, TCH], BF16, tag=f"qc{h}", bufs=1,
                                     name=f"qc{h}") for h in range(HLOC)]
                qr_loc = [pwork.tile([RHD, TCH], F32R, tag=f"qr{h}", bufs=1,
                                     name=f"qr{h}") for h in range(HLOC)]
                st[c]["qc"], st[c]["qr"] = qc_loc, qr_loc
                for h in range(HLOC):
                    # q_c (transposed [hs, t]) and k_c
                    for w_sb, dst_ap, eng in (
                        (wuq, qc_loc[h][:], "act"),
                        (wuk, kcT[h][:, tsl], "dve"),
                    ):
                        acc = pmm.tile([P, TCH], F32, tag="mm", name="mm")
                        for g in range(NNL):
                            src = cq_sb if w_sb is wuq else ckv_sb
                            nc.tensor.matmul(
                                acc[:],
                                w_sb[:, g * HLOC * HS + h * P:
                                     g * HLOC * HS + (h + 1) * P],
                                src[g][:],
                                start=(g == 0),
                                stop=(g == NNL - 1),
                            )
                        cp = (nc.scalar.copy if eng == "act"
                              else nc.vector.tensor_copy)
                        cp(dst_ap, acc[:])
                    # v in natural [t, hs] layout: 4 t-slices side by side
                    accv = pmm.tile([P, TCH], F32, tag="mm", name="mm")
                    for s in range(4):
                        for g in range(NNL):
                            nc.tensor.matmul(
                                accv[:, s * P:(s + 1) * P],
                                ckv_sb[g][:, s * P:(s + 1) * P],
                                wuv[:, g * HLOC * HS + h * P:
                                    g * HLOC * HS + (h + 1) * P],
                                start=(g == 0),
                                stop=(g == NNL - 1),
                            )
                    nc.scalar.copy(vv[h][:, tsl], accv[:])
                    # q_r raw + rope
                    accr = pmm.tile([P, TCH], F32, tag="mm", name="mm")
                    for g in range(NNL):
                        nc.tensor.matmul(
                            accr[0:RHD, :],
                            wqr[:, g * HLOC * RHD + h * RHD:
                                g * HLOC * RHD + (h + 1) * RHD],
                            cq_sb[g][:],
                            start=(g == 0),
                            stop=(g == NNL - 1),
                        )
                    rtmp2 = pwork.tile([RHD, TCH], F32, tag="rt2", name="rt2")
                    rope(qr_loc[h], slice(0, TCH), accr[0:RHD, :], rtmp2, tsl)

            def emit_attn(c):
                qc_loc, qr_loc = st[c]["qc"], st[c]["qr"]
                ah_loc = []
                st[c]["ah"] = ah_loc
                for h in range(HLOC):
                    outU = pou.tile([P, TCH], F32, tag="ou", name="ou")
                    den = pou.tile([1, TCH], F32, tag="de", name="de")
                    blocks = [(kc, ks) for kc in range(c + 1) for ks in range(4)]
                    nb = len(blocks)
                    pend = []

                    def flush_one(h=h, outU=outU, den=den, pend=pend):
                        Pt, q0, w, k0, first, last = pend.pop(0)
                        nc.tensor.matmul(
                            den[:, q0:TCH],
                            onesbf[:],
                            Pt[:, 0:w],
                            start=first,
                            stop=last,
                            skip_group_check=True,
                        )
                        nc.tensor.matmul(
                            outU[:, q0:TCH],
                            vv[h][:, k0:k0 + P],
                            Pt[:, 0:w],
                            start=first,
                            stop=last,
                            skip_group_check=True,
                        )

                    for bi, (kc, ks) in enumerate(blocks):
                        w = TCH if kc < c else TCH - P * ks
                        q0 = TCH - w
                        k0 = kc * TCH + ks * P
                        ST = pmm.tile([P, TCH], F32, tag="mm", name="mm")
                        nc.tensor.matmul(
                            ST[:, 0:w],
                            kcT[h][:, k0:k0 + P],
                            qc_loc[h][:, q0:TCH],
                            start=True,
                            stop=False,
                        )
                        nc.tensor.matmul(
                            ST[:, 0:w],
                            kr[:, k0:k0 + P],
                            qr_loc[h][:, q0:TCH],
                            start=False,
                            stop=True,
                        )
                        if kc == c:
                            nc.vector.tensor_add(
                                ST[:, 0:w], ST[:, 0:w],
                                maskbuf[:, 384:384 + w],
                            )
                        Pt = pwork.tile([P, TCH], BF16, tag="pt", bufs=6,
                                        name="pt")
                        nc.scalar.activation(Pt[:, 0:w], ST[:, 0:w], Exp,
                                             scale=SCALE)
                        pend.append((Pt, q0, w, k0, bi == 0, bi == nb - 1))
                        if len(pend) > 2:
                            flush_one()
                    while pend:
                        flush_one()

                    # normalize
                    # normalize off the PE path: recip (DVE) -> partition
                    # broadcast (Pool) -> multiply (DVE)
                    recip = pwork.tile([1, TCH], F32, tag="rc", bufs=2,
                                       name="rc")
                    nc.vector.reciprocal(recip[:], den[:])
                    bc_sb = pwork.tile([P, TCH], F32, tag="bcs", bufs=2,
                                       name="bcs")
                    nc.gpsimd.partition_broadcast(bc_sb[:], recip[:])
                    oh = pwork.tile([P, TCH], BF16, tag=f"oh{h}", bufs=1,
                                    name=f"oh{h}")
                    nc.vector.tensor_mul(oh[:], outU[:], bc_sb[:])
                    ah_loc.append(oh)

            def emit_out(c):
                tsl = slice(c * TCH, (c + 1) * TCH)
                ah_loc = st[c]["ah"]
                for cs in range(NCI):
                    acc = pmm.tile([P, TCH], F32, tag="mm", name="mm")
                    for h in range(HLOC):
                        nc.tensor.matmul(
                            acc[:],
                            wo[:, h * C + cs * P: h * C + (cs + 1) * P],
                            ah_loc[h][:],
                            start=(h == 0),
                            stop=(h == HLOC - 1),
                        )
                    ot = pwork.tile([P, TCH], BF16, tag="ot", bufs=6, name="ot")
                    cp = nc.scalar.copy if cs % 2 == 0 else nc.vector.tensor_copy
                    cp(ot[:], acc[:])
                    if cs % 2 == 0 or c == NCH - 1:
                        nc.sync.dma_start(
                            out_ext.ap()[cs * P:(cs + 1) * P, tsl], ot[:]
                        )
                    else:
                        nc.gpsimd.dma_start(
                            out=out_ext.ap()[cs * P:(cs + 1) * P, tsl],
                            in_=ot[:],
                        )

            emit_dp(0)
            emit_up(0)
            for c in range(NCH):
                if c + 1 < NCH:
                    emit_dp(c + 1)
                emit_attn(c)
                if c + 1 < NCH:
                    emit_up(c + 1)
                emit_out(c)

    nc.compile()
    return nc


def _get_nc():
    if "nc" not in _NC_CACHE:
        _NC_CACHE["nc"] = build()
    return _NC_CACHE["nc"]


def _planar(n):
    """Column permutation turning interleaved (re,im) pairs into planar
    halves: [0,2,...,n-2, 1,3,...,n-1]."""
    return list(range(0, n, 2)) + list(range(1, n, 2))


def kernel(x, freqs_cos, freqs_sin, W_dq, W_uq, W_dkv, W_uk, W_uv, W_qr, W_kr,
           W_o, trace=False, **trace_kwargs):
    nc = _get_nc()
    bf = mybir.dt.np(BF16)

    def bfT(a):
        return np.ascontiguousarray(np.asarray(a, np.float32).T).astype(bf)

    x = np.asarray(x, np.float32)
    cos = np.asarray(freqs_cos, np.float32)
    sin = np.asarray(freqs_sin, np.float32)

    xT = [bfT(x[b]) for b in range(B)]                   # [C, T]
    wdqT = bfT(W_dq)                                     # [C, NL]
    wdkvT = bfT(W_dkv)
    wkrT = bfT(W_kr)[:, _planar(RHD)]                    # [C, RHD] planar
    caT = np.ascontiguousarray(cos).astype(bf)           # [T, RHD/2]
    saT = np.ascontiguousarray(sin).astype(bf)

    pq = _planar(RHD)
    in_maps = []
    for core in range(8):
        b, r = divmod(core, 4)
        hsl = slice(r * HLOC * HS, (r + 1) * HLOC * HS)
        rsl = slice(r * HLOC * RHD, (r + 1) * HLOC * RHD)
        wqrT = bfT(W_qr[rsl])                            # [NL, 256]
        wqrT = wqrT.reshape(NL, HLOC, RHD)[:, :, pq].reshape(NL, HLOC * RHD)
        wqrT = np.ascontiguousarray(wqrT)
        in_maps.append({
            "xT": xT[b],
            "wdqT": wdqT, "wdkvT": wdkvT, "wkrT": wkrT,
            "wuqT": bfT(W_uq[hsl]),
            "wukT": bfT(W_uk[hsl]),
            "wuvT": bfT(W_uv[hsl]),
            "wqrT": wqrT,
            "woT": bfT(W_o[:, hsl]),
            "ca": caT, "sa": saT,
        })
    res = run_bass_kernel_spmd(nc, in_maps, core_ids=list(range(8)),
                               trace=trace, **trace_kwargs)
    out = np.zeros((B, T, C), dtype=np.float32)
    for core in range(8):
        b = core // 4
        out[b] += res.results[core]["out"].astype(np.float32).T
    kernel.last_result = res
    return out


# revision 46
# speedup vs baseline: 1.0021x; 1.0021x over previous
"""MLA-style attention kernel for 8 TRN2 NeuronCores.

Sharding: core c -> batch b = c//4, heads r*4..r*4+3 where r = c%4.
Each core recomputes the full latent down-projection for its batch
locally (cheaper than the ~126us AllGather the cost model charges),
then computes its 4 heads' attention and a partial output projection
summed on the host.

All layout work (x^T, every weight transpose, planar rope reordering,
cos/sin table duplication, bf16 casts) happens on the host in numpy, so
the device runs pure matmul/softmax work.  The main loop is braided
across 512-row T-chunks: the latency-bound attention row tq=c is
emitted interleaved into chunk c+1's dense down-projection matmul
groups, so PE never drains while masks (DVE) and exp (ACT) catch up;
up-projection and output projection sections separate the other
cross-engine dependencies.  Everything stays SBUF-resident; outputs are
written bf16 and upcast on the host.

Per-block attention: one bf16 content matmul (128-dim) plus one bf16
rope matmul (64-dim) accumulate S^T [k,q] in PSUM; diagonal blocks are
q-sliced (w = 512-128*ks) and get a 128-column sliding-mask add; exp
writes bf16 P^T tiles consumed by a ones-column denominator matmul and
the PV matmul (8-deep software pipeline across two interleaved heads,
both heads' denominators packed into one PSUM bank).  Normalization
runs entirely off the PE: reciprocal (DVE) -> partition_broadcast
(Pool) -> multiply (DVE).  v is computed directly in natural [t, hs]
layout for all 4 heads in one matmul group.  q_r is computed flipped
([t, rope] orientation, all heads batched: 64-row instead of 512-row
passes), roped with six 128-wide DVE ops, and transposed back with four
cheap PE transposes per head.  A short junk matmul chain on the
yet-unloaded weight tile warms the PE clock-gate ramp through the first
DMA window.
"""
import math
import numpy as np

import concourse.bass as bass
import concourse.bacc as bacc
import concourse.mybir as mybir
import concourse.tile as tile
from concourse.bass_utils import run_bass_kernel_spmd

F32 = mybir.dt.float32
F32R = mybir.dt.float32r
BF16 = mybir.dt.bfloat16
Exp = mybir.ActivationFunctionType.Exp

B, T, C = 2, 2048, 2048
H = 16
HS = 128
NL = 512
RHD = 64
HLOC = 4              # heads per core
P = 128
NNL = NL // P         # 4 latent part-tiles
NCI = C // P          # 16 c part-tiles
TCH = 512
NCH = T // TCH        # 4 chunks of T
SCALE = 1.0 / math.sqrt(HS + RHD)
NEG = -1.0e30

_NC_CACHE = {}


def build():
    nc = bacc.Bacc("TRN2", target_bir_lowering=False, debug=False, num_devices=8)

    xT_ext = nc.dram_tensor("xT", [C, T], BF16, kind="ExternalInput")
    wdqT_ext = nc.dram_tensor("wdqT", [C, NL], BF16, kind="ExternalInput")
    wdkvT_ext = nc.dram_tensor("wdkvT", [C, NL], BF16, kind="ExternalInput")
    wkrT_ext = nc.dram_tensor("wkrT", [C, RHD], BF16, kind="ExternalInput")
    wuqT_ext = nc.dram_tensor("wuqT", [NL, HLOC * HS], BF16, kind="ExternalInput")
    wukT_ext = nc.dram_tensor("wukT", [NL, HLOC * HS], BF16, kind="ExternalInput")
    wuvT_ext = nc.dram_tensor("wuvT", [NL, HLOC * HS], BF16, kind="ExternalInput")
    wqrT_ext = nc.dram_tensor("wqrT", [NL, HLOC * RHD], BF16, kind="ExternalInput")
    woT_ext = nc.dram_tensor("woT", [HLOC * HS, C], BF16, kind="ExternalInput")
    ca_ext = nc.dram_tensor("ca", [RHD, T], BF16, kind="ExternalInput")
    sa_ext = nc.dram_tensor("sa", [RHD, T], BF16, kind="ExternalInput")
    out_ext = nc.dram_tensor("out", [C, T], BF16, kind="ExternalOutput")

    bfnp = mybir.dt.np(BF16)
    onesbf_dram = nc.inline_tensor(np.ones((P, 1), dtype=bfnp), name="onesbfc")
    # transposed sliding causal mask for S^T tiles [k-sub, q]:
    # m[jj, 384 + u] = 0 iff u >= jj (else -1e30).  For the diagonal
    # kc == tq with q-slice starting at 128*ks, the slice [384:384+w]
    # allows q - 128*ks >= jj for every ks.
    m = np.full((P, 896), NEG, dtype=np.float32)
    for jj in range(P):
        m[jj, 384 + jj:] = 0.0
    masks_dram = nc.inline_tensor(m.astype(bfnp), name="maskc")

    def ci_fold(ext, width):
        """DRAM [n*128, width] viewed as [128, n, width] (n part-tiles
        stacked along the free dim)."""
        return ext.ap().rearrange("(n p) w -> p n w", p=P)

    def fold_dst(t, width):
        return t[:].rearrange("p (n w) -> p n w", w=width)

    with tile.TileContext(nc) as tc:
        with (
            tc.tile_pool(name="pers", bufs=1) as pers,
            tc.tile_pool(name="pwork", bufs=1) as pwork,
            tc.tile_pool(name="pmm", bufs=4, space="PSUM") as pmm,
            tc.tile_pool(name="pou", bufs=2, space="PSUM") as pou,
        ):
            # ---------------- persistent weights / tables ----------------
            # Load order is startup-latency-critical: interleave the x^T
            # chunk-0 pieces with W_dq pieces so the first down-proj matmul
            # can start after ~1MB of DMA, not after every weight.
            wdq = pers.tile([P, NCI * NL], BF16, tag="wdq", name="wdq")
            wdkv = pers.tile([P, NCI * NL], BF16, tag="wdkv", name="wdkv")
            wkr = pers.tile([P, NCI * RHD], BF16, tag="wkr", name="wkr")
            wuq = pers.tile([P, NNL * HLOC * HS], BF16, tag="wuq", name="wuq")
            wuk = pers.tile([P, NNL * HLOC * HS], BF16, tag="wuk", name="wuk")
            wuv = pers.tile([P, NNL * HLOC * HS], BF16, tag="wuv", name="wuv")
            wqr = pers.tile([P, NNL * HLOC * RHD], BF16, tag="wqr", name="wqr")
            wo = pers.tile([P, HLOC * C], BF16, tag="wo", name="wo")
            onesbf = pers.tile([P, 1], BF16, tag="onesbf", name="onesbf")
            maskbuf = pers.tile([P, 896], BF16, tag="maskbuf", name="maskbuf")
            ca = pers.tile([RHD, T], BF16, tag="ca", name="ca")
            sa = pers.tile([RHD, T], BF16, tag="sa", name="sa")

            def load_piece(t, ext, width, pc, npc=4):
                """Load part-tile group pc (of npc) of a folded weight."""
                n = ext.shape[0] // P
                lo, hi = pc * n // npc, (pc + 1) * n // npc
                nc.sync.dma_start(
                    fold_dst(t, width)[:, lo:hi, :],
                    ext.ap()[lo * P:hi * P, :].rearrange(
                        "(n p) w -> p n w", p=P),
                )

            def load_rest():
                nc.sync.dma_start(wkr[:].rearrange("p (n w) -> p n w", w=RHD),
                                  ci_fold(wkrT_ext, RHD))
                nc.sync.dma_start(ca[:], ca_ext.ap())
                nc.sync.dma_start(sa[:], sa_ext.ap())
                nc.sync.dma_start(onesbf[:], onesbf_dram.ap())
                nc.sync.dma_start(maskbuf[:], masks_dram.ap())
                for t, ext, width in (
                    (wuq, wuqT_ext, HLOC * HS),
                    (wuk, wukT_ext, HLOC * HS),
                    (wuv, wuvT_ext, HLOC * HS),
                    (wqr, wqrT_ext, HLOC * RHD),
                    (wo, woT_ext, C),
                ):
                    nc.sync.dma_start(fold_dst(t, width),
                                      ci_fold(ext, width))

            # persistent per-head K/V state + shared rope key
            kcT = [pers.tile([P, T], BF16, tag=f"kcT{h}", name=f"kcT{h}")
                   for h in range(HLOC)]
            vv = [pers.tile([P, T], BF16, tag=f"vv{h}", name=f"vv{h}")
                  for h in range(HLOC)]
            kr = pers.tile([RHD, T], BF16, tag="kr", name="kr")

            def rope(dst, dst_sl, raw, tmp, sl):
                """dst[:, dst_sl] = rope(raw) with planar re/im halves."""
                nc.vector.tensor_mul(tmp[0:32, :], raw[32:64, :], sa[32:64, sl])
                nc.vector.tensor_mul(tmp[32:64, :], raw[32:64, :], ca[32:64, sl])
                nc.vector.tensor_mul(dst[0:32, dst_sl], raw[0:32, :], ca[0:32, sl])
                nc.vector.tensor_mul(dst[32:64, dst_sl], raw[0:32, :], sa[0:32, sl])
                nc.vector.tensor_sub(
                    dst[0:32, dst_sl], dst[0:32, dst_sl], tmp[0:32, :]
                )
                nc.vector.tensor_add(
                    dst[32:64, dst_sl], dst[32:64, dst_sl], tmp[32:64, :]
                )

            # ---------------- chunk-pipelined main loop -------------------
            # Emission is braided across chunks so that the latency-bound
            # attention row tq=c executes while the dense down-proj of
            # chunk c+1 keeps every other engine's queue drained:
            #   dp(0) up(0) [dp(1) attn(0) up(1) out(0)] [dp(2) attn(1) ...
            st = [dict() for _ in range(NCH)]

            def emit_dp(c):
                tsl = slice(c * TCH, (c + 1) * TCH)
                # prefetch x^T for chunk c+1 one full section group ahead
                if c == 0:
                    # junk matmuls on the not-yet-loaded weight tile keep
                    # the PE busy through the first DMA window so the
                    # clock-gate ramp is warm when real matmuls start
                    for _ in range(5):
                        warm = pmm.tile([P, 256], F32, tag="mm", name="mm")
                        nc.tensor.matmul(warm[:], kcT[0][:, 0:P],
                                         kcT[0][:, 0:256],
                                         start=True, stop=True,
                                         skip_group_check=True)
                if c == 0:
                    st[0]["xt"] = pwork.tile([P, NCI * TCH], BF16, tag="xt",
                                             bufs=2, name="xt")
                    for pc in range(8):
                        lo, hi = pc * 2, (pc + 1) * 2
                        nc.sync.dma_start(
                            fold_dst(st[0]["xt"], TCH)[:, lo:hi, :],
                            xT_ext.ap()[lo * P:hi * P, 0:TCH].rearrange(
                                "(n p) w -> p n w", p=P),
                        )
                        load_piece(wdq, wdqT_ext, NL, pc, npc=8)
                    for pc in range(4):
                        load_piece(wdkv, wdkvT_ext, NL, pc)
                    load_rest()
                if c + 1 < NCH:
                    nxt = pwork.tile([P, NCI * TCH], BF16, tag="xt", bufs=2,
                                     name="xt")
                    st[c + 1]["xt"] = nxt
                    nc.sync.dma_start(
                        fold_dst(nxt, TCH),
                        xT_ext.ap()[:, (c + 1) * TCH:(c + 2) * TCH].rearrange(
                            "(n p) w -> p n w", p=P),
                    )
                xt = st[c]["xt"]
                cq_sb = [pwork.tile([P, TCH], BF16, tag=f"cq{g}", bufs=1,
                                    name=f"cq{g}") for g in range(NNL)]
                ckv_sb = [pwork.tile([P, TCH], BF16, tag=f"ckv{g}", bufs=1,
                                     name=f"ckv{g}") for g in range(NNL)]
                st[c]["cq"], st[c]["ckv"] = cq_sb, ckv_sb
                for w_sb, dst in ((wdq, cq_sb), (wdkv, ckv_sb)):
                    for g in range(NNL):
                        acc = pmm.tile([P, TCH], F32, tag="mm", name="mm")
                        for ci in range(NCI):
                            nc.tensor.matmul(
                                acc[:],
                                w_sb[:, ci * NL + g * P: ci * NL + (g + 1) * P],
                                xt[:, ci * TCH:(ci + 1) * TCH],
                                start=(ci == 0),
                                stop=(ci == NCI - 1),
                            )
                        cp = nc.scalar.copy if g % 2 == 0 else nc.vector.tensor_copy
                        cp(dst[g][:], acc[:])
                acck = pmm.tile([P, TCH], F32, tag="mm", name="mm")
                for ci in range(NCI):
                    nc.tensor.matmul(
                        acck[0:RHD, :],
                        wkr[:, ci * RHD:(ci + 1) * RHD],
                        xt[:, ci * TCH:(ci + 1) * TCH],
                        start=(ci == 0),
                        stop=(ci == NCI - 1),
                    )
                rtmp = pwork.tile([RHD, TCH], F32, tag="rtmp", name="rtmp")
                rope(kr, tsl, acck[0:RHD, :], rtmp, tsl)

            def emit_up(c):
                tsl = slice(c * TCH, (c + 1) * TCH)
                cq_sb, ckv_sb = st[c]["cq"], st[c]["ckv"]
                qc_loc = [pwork.tile([P, TCH], BF16, tag=f"qc{h}", bufs=1,
                                     name=f"qc{h}") for h in range(HLOC)]
                qr_loc = [pwork.tile([RHD, TCH], F32R, tag=f"qr{h}", bufs=1,
                                     name=f"qr{h}") for h in range(HLOC)]
                st[c]["qc"], st[c]["qr"] = qc_loc, qr_loc
                for h in range(HLOC):
                    # q_c (transposed [hs, t]) and k_c
                    for w_sb, dst_ap, eng in (
                        (wuq, qc_loc[h][:], "act"),
                        (wuk, kcT[h][:, tsl], "dve"),
                    ):
                        acc = pmm.tile([P, TCH], F32, tag="mm", name="mm")
                        for g in range(NNL):
                            src = cq_sb if w_sb is wuq else ckv_sb
                            nc.tensor.matmul(
                                acc[:],
                                w_sb[:, g * HLOC * HS + h * P:
                                     g * HLOC * HS + (h + 1) * P],
                                src[g][:],
                                start=(g == 0),
                                stop=(g == NNL - 1),
                            )
                        cp = (nc.scalar.copy if eng == "act"
                              else nc.vector.tensor_copy)
                        cp(dst_ap, acc[:])
                    # v in natural [t, hs] layout: 4 t-slices side by side
                    accv = pmm.tile([P, TCH], F32, tag="mm", name="mm")
                    for s in range(4):
                        for g in range(NNL):
                            nc.tensor.matmul(
                                accv[:, s * P:(s + 1) * P],
                                ckv_sb[g][:, s * P:(s + 1) * P],
                                wuv[:, g * HLOC * HS + h * P:
                                    g * HLOC * HS + (h + 1) * P],
                                start=(g == 0),
                                stop=(g == NNL - 1),
                            )
                    nc.scalar.copy(vv[h][:, tsl], accv[:])
                    # q_r raw + rope
                    accr = pmm.tile([P, TCH], F32, tag="mm", name="mm")
                    for g in range(NNL):
                        nc.tensor.matmul(
                            accr[0:RHD, :],
                            wqr[:, g * HLOC * RHD + h * RHD:
                                g * HLOC * RHD + (h + 1) * RHD],
                            cq_sb[g][:],
                            start=(g == 0),
                            stop=(g == NNL - 1),
                        )
                    rtmp2 = pwork.tile([RHD, TCH], F32, tag="rt2", name="rt2")
                    rope(qr_loc[h], slice(0, TCH), accr[0:RHD, :], rtmp2, tsl)

            def emit_attn(c):
                qc_loc, qr_loc = st[c]["qc"], st[c]["qr"]
                ah_loc = []
                st[c]["ah"] = ah_loc
                for h in range(HLOC):
                    outU = pou.tile([P, TCH], F32, tag="ou", name="ou")
                    den = pou.tile([1, TCH], F32, tag="de", name="de")
                    blocks = [(kc, ks) for kc in range(c + 1) for ks in range(4)]
                    nb = len(blocks)
                    pend = []

                    def flush_one(h=h, outU=outU, den=den, pend=pend):
                        Pt, q0, w, k0, first, last = pend.pop(0)
                        nc.tensor.matmul(
                            den[:, q0:TCH],
                            onesbf[:],
                            Pt[:, 0:w],
                            start=first,
                            stop=last,
                            skip_group_check=True,
                        )
                        nc.tensor.matmul(
                            outU[:, q0:TCH],
                            vv[h][:, k0:k0 + P],
                            Pt[:, 0:w],
                            start=first,
                            stop=last,
                            skip_group_check=True,
                        )

                    for bi, (kc, ks) in enumerate(blocks):
                        w = TCH if kc < c else TCH - P * ks
                        q0 = TCH - w
                        k0 = kc * TCH + ks * P
                        ST = pmm.tile([P, TCH], F32, tag="mm", name="mm")
                        nc.tensor.matmul(
                            ST[:, 0:w],
                            kcT[h][:, k0:k0 + P],
                            qc_loc[h][:, q0:TCH],
                            start=True,
                            stop=False,
                        )
                        nc.tensor.matmul(
                            ST[:, 0:w],
                            kr[:, k0:k0 + P],
                            qr_loc[h][:, q0:TCH],
                            start=False,
                            stop=True,
                        )
                        if kc == c:
                            nc.vector.tensor_add(
                                ST[:, 0:w], ST[:, 0:w],
                                maskbuf[:, 384:384 + w],
                            )
                        Pt = pwork.tile([P, TCH], BF16, tag="pt", bufs=6,
                                        name="pt")
                        nc.scalar.activation(Pt[:, 0:w], ST[:, 0:w], Exp,
                                             scale=SCALE)
                        pend.append((Pt, q0, w, k0, bi == 0, bi == nb - 1))
                        if len(pend) > 2:
                            flush_one()
                    while pend:
                        flush_one()

                    # normalize
                    # normalize off the PE path: recip (DVE) -> partition
                    # broadcast (Pool) -> multiply (DVE)
                    recip = pwork.tile([1, TCH], F32, tag="rc", bufs=2,
                                       name="rc")
                    nc.vector.reciprocal(recip[:], den[:])
                    bc_sb = pwork.tile([P, TCH], F32, tag="bcs", bufs=2,
                                       name="bcs")
                    nc.gpsimd.partition_broadcast(bc_sb[:], recip[:])
                    oh = pwork.tile([P, TCH], BF16, tag=f"oh{h}", bufs=1,
                                    name=f"oh{h}")
                    nc.vector.tensor_mul(oh[:], outU[:], bc_sb[:])
                    ah_loc.append(oh)

            def emit_out(c):
                tsl = slice(c * TCH, (c + 1) * TCH)
                ah_loc = st[c]["ah"]
                for cs in range(NCI):
                    acc = pmm.tile([P, TCH], F32, tag="mm", name="mm")
                    for h in range(HLOC):
                        nc.tensor.matmul(
                            acc[:],
                            wo[:, h * C + cs * P: h * C + (cs + 1) * P],
                            ah_loc[h][:],
                            start=(h == 0),
                            stop=(h == HLOC - 1),
                        )
                    ot = pwork.tile([P, TCH], BF16, tag="ot", bufs=6, name="ot")
                    cp = nc.scalar.copy if cs % 2 == 0 else nc.vector.tensor_copy
                    cp(ot[:], acc[:])
                    if cs % 2 == 0 or c == NCH - 1:
                        nc.sync.dma_start(
                            out_ext.ap()[cs * P:(cs + 1) * P, tsl], ot[:]
                        )
                    else:
                        nc.gpsimd.dma_start(
                            out=out_ext.ap()[cs * P:(cs + 1) * P, tsl],
                            in_=ot[:],
                        )

            emit_dp(0)
            emit_up(0)
            for c in range(NCH):
                if c + 1 < NCH:
                    emit_dp(c + 1)
                emit_attn(c)
                if c + 1 < NCH:
                    emit_up(c + 1)
                emit_out(c)

    nc.compile()
    return nc


def _get_nc():
    if "nc" not in _NC_CACHE:
        _NC_CACHE["nc"] = build()
    return _NC_CACHE["nc"]


def _planar(n):
    """Column permutation turning interleaved (re,im) pairs into planar
    halves: [0,2,...,n-2, 1,3,...,n-1]."""
    return list(range(0, n, 2)) + list(range(1, n, 2))


def kernel(x, freqs_cos, freqs_sin, W_dq, W_uq, W_dkv, W_uk, W_uv, W_qr, W_kr,
           W_o, trace=False, **trace_kwargs):
    nc = _get_nc()
    bf = mybir.dt.np(BF16)

    def bfT(a):
        return np.ascontiguousarray(np.asarray(a, np.float32).T).astype(bf)

    x = np.asarray(x, np.float32)
    cos = np.asarray(freqs_cos, np.float32)
    sin = np.asarray(freqs_sin, np.float32)

    xT = [bfT(x[b]) for b in range(B)]                   # [C, T]
    wdqT = bfT(W_dq)                                     # [C, NL]
    wdkvT = bfT(W_dkv)
    wkrT = bfT(W_kr)[:, _planar(RHD)]                    # [C, RHD] planar
    caT = np.ascontiguousarray(cos).astype(bf)           # [T, RHD/2]
    saT = np.ascontiguousarray(sin).astype(bf)

    pq = _planar(RHD)
    in_maps = []
    for core in range(8):
        b, r = divmod(core, 4)
        hsl = slice(r * HLOC * HS, (r + 1) * HLOC * HS)
        rsl = slice(r * HLOC * RHD, (r + 1) * HLOC * RHD)
        wqrT = bfT(W_qr[rsl])                            # [NL, 256]
        wqrT = wqrT.reshape(NL, HLOC, RHD)[:, :, pq].reshape(NL, HLOC * RHD)
        wqrT = np.ascontiguousarray(wqrT)
        in_maps.append({
            "xT": xT[b],
            "wdqT": wdqT, "wdkvT": wdkvT, "wkrT": wkrT,
            "wuqT": bfT(W_uq[hsl]),
            "wukT": bfT(W_uk[hsl]),
            "wuvT": bfT(W_uv[hsl]),
            "wqrT": wqrT,
            "woT": bfT(W_o[:, hsl]),
            "ca": caT, "sa": saT,
        })
    res = run_bass_kernel_spmd(nc, in_maps, core_ids=list(range(8)),
                               trace=trace, **trace_kwargs)
    out = np.zeros((B, T, C), dtype=np.float32)
    for core in range(8):
        b = core // 4
        out[b] += res.results[core]["out"].astype(np.float32).T
    kernel.last_result = res
    return out
